# revision 1
# baseline (speedup 1.0000x reference)
"""NT-Xent loss kernel for Trainium2 (8 NeuronCores, SPMD row-sharded).

Reference computation (N=4096, D=256, T=0.5):
    zi, zj = l2norm(z_i), l2norm(z_j); reps = concat([zi, zj])  # [2N, D]
    sim = reps @ reps.T
    lse_a = logsumexp over row a of sim/T with the diagonal excluded
    pos_a = sim[a, a+-N]
    loss = mean(lse_a - pos_a/T)

Sharding: every core holds the full (bf16-cast) reps for the matmul
column side; core c additionally gets its 1024-row slice `zb` to use as
the stationary (row) side.  Each core produces, for its 1024 rows,
S'_a = sum_b!=a exp(2*sim_ab) plus (replicated, cheap) the positive-pair
cosines.  Host does the final ln + mean (the "all-reduce" step).

Device pipeline per core:
  1. SWDGE cast-DMA loads HBM fp32 -> SBUF bf16 (z_i, z_j, zb).
  2. Row norms: tensor_tensor_reduce squares -> ssq; invn = exp(-0.5*ln(ssq)).
  3. Normalize in place (tensor_scalar per tile).
  4. Transpose normalized reps into repsT [128, 2, 8192] via PE
     matmul-with-identity (out = tile.T), PSUM -> SBUF copy-cast to bf16.
  5. Main loop: sim block [128, 512] = znbT.T @ repsT (2 K-halves into one
     PSUM bank); ScalarE exp(scale=2) with accum_out giving row partials.
  6. Diagonal term exp(2*|zn_a|^2) subtracted from the row sums.
"""

import sys
import os

for _p in ("/opt/trn_rl_repo",):
    if _p not in sys.path:
        sys.path.insert(0, _p)

import numpy as np
from contextlib import ExitStack

import concourse.bass as bass
import concourse.tile as tile
from concourse import mybir
from concourse.masks import make_identity
from concourse.vector_clock import ScopedClock as _ScopedClock


def _patched_drain_and_barrier(self, tick_clock, wait_clock):
    """Tile's closing drain carries one sem-wait per DMA lane used, but this
    walrus build only accepts a single sync wait on a Drain (CTRL-NO)
    lowering ("Too many sync wait commands").  Split the waits across a
    chain of drains (sequential on SP, so semantics are unchanged)."""
    nc = self.nc
    drain_inst = nc.sync.drain()
    wait_clock.add_sem_waits(
        drain_inst.ins, _ScopedClock({None: tick_clock.global_clock})
    )
    si = drain_inst.ins.sync_info
    if si is not None:
        waits = list(si.on_wait or [])
        if len(waits) > 1:
            import bass_rust as _br

            si.on_wait = waits[:1]
            for w in waits[1:]:
                d2 = nc.sync.drain()
                d2.ins.sync_info = _br.SyncInfo(on_wait=[w], on_update=[])
    nc.all_engine_barrier()
    assert self.sems is not None
    popped = nc._tile_sem_poison_stack.pop()
    assert popped is self._sem_poison
    nc.clear_and_free_semaphores(list(self.sems.allocated().values()))
    nc.all_engine_barrier()


tile.TileContext._drain_and_barrier = _patched_drain_and_barrier

_orig_lower_ordered = tile.TileContext._lower_ordered_insts


def _split_multiwaits_and_lower(self, ordered):
    """Same walrus limitation as above, for scheduled compute/DMA
    instructions: hoist all but one sync wait onto single-wait NoOps that
    precede the instruction on its own engine."""
    nc = self.nc
    for insts in ordered.values():
        if not any(
            inst.sync_info is not None and len(inst.sync_info.on_wait or []) > 1
            for inst in insts
        ):
            continue
        out = []
        for inst in insts:
            si = inst.sync_info
            waits = list(si.on_wait) if si is not None and si.on_wait else []
            if len(waits) > 1 and getattr(inst, "engine", None) is not None:
                for w in waits[:-1]:
                    out.append(
                        mybir.InstNoOp(
                            name=nc.get_next_instruction_name(),
                            sync_info=mybir.SyncInfo(on_wait=[w], on_update=[]),
                            bass_nofuse=True,
                            engine=inst.engine,
                        )
                    )
                si.on_wait = waits[-1:]
            out.append(inst)
        insts[:] = out
    return _orig_lower_ordered(self, ordered)


tile.TileContext._lower_ordered_insts = _split_multiwaits_and_lower

N_CORES = 8
N_FULL = 4096
D_FULL = 256

f32 = mybir.dt.float32
bf16 = mybir.dt.bfloat16
ALU = mybir.AluOpType
AF = mybir.ActivationFunctionType


def build_bass(N=N_FULL, D=D_FULL, n_cores=N_CORES):
    n2 = 2 * N
    R = n2 // n_cores          # rows per core
    TF = n2 // 128             # full 128-row tiles (64)
    TI = N // 128              # z_i tiles (32)
    TB = R // 128              # per-core row tiles (8)
    KH = D // 128              # contraction halves (2)
    CBW = 512                  # similarity column-block width / PSUM bank
    CH = min(8, TI)            # pipeline chunk: tiles per load/norm/transpose step
    fp8 = mybir.dt.float8e4

    assert R % 128 == 0 and D == 256 and n2 % CBW == 0 and TI % CH == 0

    nc = bass.Bass()
    z_i = nc.declare_dram_parameter("z_i", [N, D], f32, isOutput=False)
    z_j = nc.declare_dram_parameter("z_j", [N, D], f32, isOutput=False)
    zb = nc.declare_dram_parameter("zb", [R, D], f32, isOutput=False)
    lse_out = nc.declare_dram_parameter("lse_in", [128, TB], f32, isOutput=True)
    pos_out = nc.declare_dram_parameter("pos2", [128, TI], f32, isOutput=True)

    with ExitStack() as ctx:
        tc = ctx.enter_context(tile.TileContext(nc))
        big = ctx.enter_context(tc.tile_pool(name="big", bufs=1))
        escr = ctx.enter_context(tc.tile_pool(name="escr", bufs=2))
        pmm = ctx.enter_context(tc.tile_pool(name="pmm", bufs=2, space="PSUM"))

        ident = big.tile([128, 128], bf16)
        make_identity(nc, ident)

        zf = big.tile([128, TF, D], bf16)    # all reps rows, bf16
        zbn = big.tile([128, TB, D], bf16)   # this core's rows, bf16
        sq3 = big.tile([128, TF, D], bf16)
        sqb = big.tile([128, TB, D], bf16)
        ssq = big.tile([128, TF + TB], f32)
        lnssq = big.tile([128, TF + TB], f32)
        invn = big.tile([128, TF + TB], f32)
        repsT = big.tile([128, KH, n2], fp8)
        znbT = big.tile([128, KH, R], fp8)

        zi_r = z_i[:, :].rearrange("(t p) d -> p t d", p=128)
        zj_r = z_j[:, :].rearrange("(t p) d -> p t d", p=128)

        def chunk_pipeline(dst, src_r, t0, ntiles, zsq, ssq0):
            """load chunk -> sumsq -> invn -> normalize -> transpose+cast.

            dst: natural-layout bf16 tile region ([128, *, D] at tile offset t0)
            src_r: rearranged DRAM AP (or None if already loaded)
            zsq: square scratch region; ssq0: column offset into ssq/invn
            tT: transposed fp8 destination written at column t0*128..
            """
            sl = slice(t0, t0 + ntiles)
            qsl = slice(ssq0, ssq0 + ntiles)
            if src_r is not None:
                nc.gpsimd.dma_start(out=dst[:, sl, :], in_=src_r)
            nc.vector.tensor_mul(
                out=zsq[:, sl, :], in0=dst[:, sl, :], in1=dst[:, sl, :]
            )
            nc.vector.reduce_sum(
                out=ssq[:, qsl], in_=zsq[:, sl, :], axis=mybir.AxisListType.X
            )
            nc.scalar.activation(out=lnssq[:, qsl], in_=ssq[:, qsl], func=AF.Ln)
            nc.scalar.activation(
                out=invn[:, qsl], in_=lnssq[:, qsl], func=AF.Exp, scale=-0.5
            )
            for j in range(ntiles):
                nc.vector.tensor_scalar_mul(
                    out=dst[:, t0 + j, :], in0=dst[:, t0 + j, :],
                    scalar1=invn[:, ssq0 + j : ssq0 + j + 1],
                )

        def chunk_transpose(dst_T, src, t0, ntiles, tcol0):
            # both K-halves of this chunk into one 4-bank PSUM tile, then two
            # fp32->fp8 cast copies into the transposed layout
            pt = pmm.tile([128, 2048], f32, tag="ps")
            for h in range(KH):
                for j in range(ntiles):
                    nc.tensor.matmul(
                        out=pt[:, h * 1024 + j * 128 : h * 1024 + (j + 1) * 128],
                        lhsT=src[:, t0 + j, h * 128 : (h + 1) * 128],
                        rhs=ident,
                        start=True, stop=True,
                    )
            for h in range(KH):
                nc.vector.tensor_copy(
                    out=dst_T[:, h, tcol0 : tcol0 + ntiles * 128],
                    in_=pt[:, h * 1024 : h * 1024 + ntiles * 128],
                )

        # ---- per-core row block first: it gates every main-loop matmul ----
        nc.gpsimd.dma_start(
            out=zbn[:, :, :], in_=zb[:, :].rearrange("(t p) d -> p t d", p=128)
        )
        chunk_pipeline(zbn, None, 0, TB, sqb, TF)
        for g0 in range(0, TB, CH):
            gn = min(CH, TB - g0)
            chunk_transpose(znbT, zbn, g0, gn, g0 * 128)

        # ---- full reps, pipelined in CH-tile chunks ----
        for c0 in range(0, TF, CH):
            src = (
                zi_r[:, c0 : c0 + CH, :]
                if c0 + CH <= TI
                else zj_r[:, c0 - TI : c0 - TI + CH, :]
            )
            chunk_pipeline(zf, src, c0, CH, sq3, c0)
            chunk_transpose(repsT, zf, c0, CH, c0 * 128)

        # ---- main loop: [128, 2048] sim super-blocks, fp8 DoubleRow ----
        # One DoubleRow matmul per 512-column bank folds both K-halves
        # (contraction 256 via 2 fp8 weights/cell); one wide ACTIVATE does
        # exp(2*sim) plus the row-sum accumulator.
        SBW = min(2048, n2)
        NSB = n2 // SBW
        MMW = SBW // CBW
        Spart = big.tile([128, TB, NSB], f32)
        for rb in range(TB):
            for sb in range(NSB):
                ps = pmm.tile([128, SBW], f32, tag="ps")
                for j in range(MMW):
                    c0 = (sb * MMW + j) * CBW
                    nc.tensor.matmul(
                        out=ps[:, j * CBW : (j + 1) * CBW],
                        lhsT=znbT[:, :, rb * 128 : (rb + 1) * 128],
                        rhs=repsT[:, :, c0 : c0 + CBW],
                        start=True, stop=True,
                        perf_mode=mybir.MatmulPerfMode.DoubleRow,
                    )
                e = escr.tile([128, SBW], bf16, tag="e")
                nc.scalar.activation(
                    out=e, in_=ps, func=AF.Exp, scale=2.0,
                    accum_out=Spart[:, rb, sb : sb + 1],
                )

        # ---- positive pairs: pos2[p, t] = 2 * <zn_i[t*128+p], zn_j[t*128+p]> ----
        posr = big.tile([128, TI], f32)
        nc.vector.tensor_mul(
            out=sq3[:, 0:TI, :], in0=zf[:, 0:TI, :], in1=zf[:, TI:TF, :]
        )
        nc.vector.reduce_sum(
            out=posr, in_=sq3[:, 0:TI, :], axis=mybir.AxisListType.X
        )
        pos2 = big.tile([128, TI], f32)
        nc.vector.tensor_scalar_mul(out=pos2, in0=posr, scalar1=2.0)
        nc.sync.dma_start(out=pos_out[:, :], in_=pos2)

        # ---- diagonal terms: consistent with the fp8 matmul inputs ----
        # d_a = |fp8(zn_a)|^2 via an fp8 square in the transposed layout is
        # awkward; bf16 zn is close enough (residual ~1e-5 on the final ln).
        dacc = big.tile([128, TB], f32)
        nc.vector.tensor_mul(out=sqb[:, :, :], in0=zbn[:, :, :], in1=zbn[:, :, :])
        nc.vector.reduce_sum(out=dacc, in_=sqb[:, :, :], axis=mybir.AxisListType.X)
        expd = big.tile([128, TB], f32)
        nc.scalar.activation(out=expd, in_=dacc, func=AF.Exp, scale=2.0)

        # ---- S' = sum - diag, ship out ----
        S_t = big.tile([128, TB], f32)
        nc.vector.reduce_sum(out=S_t, in_=Spart[:, :, :], axis=mybir.AxisListType.X)
        lse_in_t = big.tile([128, TB], f32)
        nc.vector.tensor_sub(out=lse_in_t, in0=S_t, in1=expd)
        nc.sync.dma_start(out=lse_out[:, :], in_=lse_in_t)

    return nc


_NC_CACHE = {}


def _get_nc(N=N_FULL, D=D_FULL):
    key = (N, D)
    if key not in _NC_CACHE:
        _NC_CACHE[key] = build_bass(N, D)
    return _NC_CACHE[key]


def make_in_maps(z_i, z_j, n_cores=N_CORES):
    z_i = np.ascontiguousarray(z_i, dtype=np.float32)
    z_j = np.ascontiguousarray(z_j, dtype=np.float32)
    reps = np.concatenate([z_i, z_j], axis=0)
    R = reps.shape[0] // n_cores
    return [
        {
            "z_i": z_i,
            "z_j": z_j,
            "zb": np.ascontiguousarray(reps[c * R : (c + 1) * R]),
        }
        for c in range(n_cores)
    ]


def assemble(results, N=N_FULL):
    """Host-side gather + final ln/mean ("all-reduce the mean loss")."""
    n2 = 2 * N
    lse_in = np.stack([np.asarray(r["lse_in"], dtype=np.float64) for r in results])
    # lse_in[c, p, rb] -> row c*R + rb*128 + p
    lse_vec = lse_in.transpose(0, 2, 1).reshape(n2)
    pos2 = np.asarray(results[0]["pos2"], dtype=np.float64)
    pos_vec = pos2.T.reshape(N)  # [p, t] -> row t*128+p
    lse = np.log(lse_vec)
    loss = np.mean(lse - np.concatenate([pos_vec, pos_vec]))
    return np.float32(loss)


def _run(z_i, z_j, trace=False, tmpdir=None, **spmd_kwargs):
    from concourse.bass_utils import run_bass_kernel_spmd

    N, D = z_i.shape
    nc = _get_nc(N, D)
    in_maps = make_in_maps(z_i, z_j)
    out = run_bass_kernel_spmd(
        nc, in_maps, list(range(N_CORES)), trace=trace, tmpdir=tmpdir, **spmd_kwargs
    )
    return assemble(out.results, N), out


def kernel(z_i, z_j):
    loss, _ = _run(np.asarray(z_i), np.asarray(z_j))
    return loss


if __name__ == "__main__":
    rng = np.random.default_rng(0)
    z_i = rng.standard_normal((N_FULL, D_FULL), dtype=np.float32)
    z_j = rng.standard_normal((N_FULL, D_FULL), dtype=np.float32)
    print(kernel(z_i, z_j))

